# revision 6
# baseline (speedup 1.0000x reference)
"""Trainium2 Bass kernel for the ConditionalDDPM forward-diffusion problem.

Computes  xt = sqrt(alpha_bar[t]) * images + sqrt(1 - alpha_bar[t]) * e
for B=65536 images of shape (1, 28, 28), t in [0, 1000).

Strategy (pure data parallel, 8 NeuronCores):
  - Shard images/e/t along batch: 8192 samples per core.
  - The 1000-entry (sqrt(ab), sqrt(1-ab)) table is a host-computed constant
    (input-independent), replicated to every core as a small DRAM input.
  - On device, the per-sample scalars are fetched with a single SWDGE
    dma_gather (8192 descriptors, 256B table rows) which lands them directly
    in per-partition layout [128, 64, 64].
  - Main stream: 16 groups of [128 partitions x 4 samples x 784 pixels]
    (1.57 MB DMAs).  Per 128-sample unit:
        ACT:  u  = a * x          (activation Copy with per-partition scale)
        DVE:  xt = (b * e) + u    (scalar_tensor_tensor, per-partition scalar)
    Everything hides under the ~77 MB/core HBM stream.
"""

import sys

if "/opt/trn_rl_repo" not in sys.path:
    sys.path.insert(0, "/opt/trn_rl_repo")

import numpy as np

B = 65536
T = 1000
BETA_1 = 1e-4
BETA_T = 0.02
N_CORES = 8
NS = B // N_CORES  # samples per core = 8192
PIX = 784
K = 4  # 128-sample units per DMA group

_cache = {}


def alpha_tables() -> np.ndarray:
    """[T, 64] f32: row t = [sqrt(alpha_bar[t]), sqrt(1-alpha_bar[t]), 0...].

    Rows are padded to 256 B because dma_gather requires elem_size % 256B == 0.
    Computed exactly as the reference does (f32 arithmetic).
    """
    slope = np.float32((BETA_T - BETA_1) / (T - 1))
    betas = np.float32(BETA_1) + slope * np.arange(T, dtype=np.float32)
    ab = np.cumprod((np.float32(1.0) - betas).astype(np.float32)).astype(np.float32)
    tab = np.zeros((T, 64), dtype=np.float32)
    tab[:, 0] = np.sqrt(ab).astype(np.float32)
    tab[:, 1] = np.sqrt((np.float32(1.0) - ab).astype(np.float32)).astype(np.float32)
    return tab


def build_program(ns: int = NS, k: int = K):
    """Build the per-core Bass program (same NEFF on all 8 cores)."""
    from concourse import bacc, mybir
    import concourse.tile as tile

    assert ns % (128 * k) == 0 and ns % 16 == 0
    n_units = ns // 128
    n_io = ns // (128 * k)
    f32 = mybir.dt.float32

    nc = bacc.Bacc(
        "TRN2",
        target_bir_lowering=False,
        debug=False,
        enable_asserts=False,
        num_devices=N_CORES,
    )
    x = nc.dram_tensor("x", [ns, PIX], f32, kind="ExternalInput").ap()
    y = nc.dram_tensor("y", [ns, PIX], f32, kind="ExternalInput").ap()
    tt = nc.dram_tensor("t", [ns], mybir.dt.int32, kind="ExternalInput").ap()
    table = nc.dram_tensor("table", [T, 64], f32, kind="ExternalInput").ap()
    out = nc.dram_tensor("out", [ns, PIX], f32, kind="ExternalOutput").ap()

    # sample s = io*(128*k) + kk*128 + p  lives at (group io, partition p, slot kk)
    x_v = x.rearrange("(io k p) m -> io p k m", io=n_io, k=k, p=128)
    y_v = y.rearrange("(io k p) m -> io p k m", io=n_io, k=k, p=128)
    o_v = out.rearrange("(io k p) m -> io p k m", io=n_io, k=k, p=128)
    # wrapped index view: slot j of the gather reads idxs[j%16, j//16] = t[j]
    t_wrapped = tt.rearrange("(m r) -> r m", r=16)

    with tile.TileContext(nc) as tc:
        with (
            tc.tile_pool(name="xs", bufs=4) as xpool,
            tc.tile_pool(name="ys", bufs=4) as ypool,
            tc.tile_pool(name="singles", bufs=1) as singles,
        ):
            # ---- per-sample scalar prep (runs once, hides under first loads) ----
            tw = singles.tile([128, ns // 16], mybir.dt.int32)
            nc.gpsimd.dma_start(out=tw[0:16, :], in_=t_wrapped)
            # dma_gather's Q7 cores each read their own 16-partition group:
            # replicate the wrapped indices to groups 1..7.
            for g in range(1, 8):
                nc.gpsimd.dma_start(out=tw[16 * g : 16 * (g + 1), :], in_=tw[0:16, :])
            # int32 -> int16 (t < 1000 fits): keep low halves of the int32 pairs
            idx16 = singles.tile([128, ns // 16], mybir.dt.int16)
            tw16 = tw[:].bitcast(mybir.dt.int16).rearrange("p (m two) -> p m two", two=2)
            nc.vector.tensor_copy(out=idx16[:], in_=tw16[:, :, 0])
            # gath[p, i, 0:2] = table[t[i*128 + p], 0:2]
            # (split into 1024-index chunks: larger single gathers crash the
            # SWDGE path at runtime — 2048+ fails, 1024 is reliable)
            gath = singles.tile([128, n_units, 64], f32)
            chunk = min(ns, 1024)
            for c in range(ns // chunk):
                nc.gpsimd.dma_gather(
                    out_ap=gath[:, c * (chunk // 128) : (c + 1) * (chunk // 128), :],
                    in_ap=table,
                    idxs_ap=idx16[:, c * (chunk // 16) : (c + 1) * (chunk // 16)],
                    num_idxs=chunk,
                    num_idxs_reg=chunk,
                    elem_size=64,
                )

            # ---- main stream ----
            for io in range(n_io):
                xt = xpool.tile([128, k, PIX], f32)
                nc.sync.dma_start(out=xt[:], in_=x_v[io, :, :, :])
                yt = ypool.tile([128, k, PIX], f32)
                nc.sync.dma_start(out=yt[:], in_=y_v[io, :, :, :])
                for kk in range(k):
                    i = io * k + kk
                    nc.scalar.activation(
                        out=xt[:, kk, :],
                        in_=xt[:, kk, :],
                        func=mybir.ActivationFunctionType.Copy,
                        scale=gath[:, i, 0:1],
                    )
                    nc.vector.scalar_tensor_tensor(
                        out=xt[:, kk, :],
                        in0=yt[:, kk, :],
                        scalar=gath[:, i, 1:2],
                        in1=xt[:, kk, :],
                        op0=mybir.AluOpType.mult,
                        op1=mybir.AluOpType.add,
                    )
                nc.scalar.dma_start(out=o_v[io, :, :, :], in_=xt[:])

    nc.compile()
    return nc


def make_in_maps(images, e, t):
    x = np.ascontiguousarray(np.asarray(images, dtype=np.float32).reshape(B, PIX))
    yy = np.ascontiguousarray(np.asarray(e, dtype=np.float32).reshape(B, PIX))
    tt = np.ascontiguousarray(np.asarray(t, dtype=np.int32).reshape(B))
    tab = alpha_tables()
    in_maps = []
    for c in range(N_CORES):
        sl = slice(c * NS, (c + 1) * NS)
        in_maps.append(
            {
                "x": np.ascontiguousarray(x[sl]),
                "y": np.ascontiguousarray(yy[sl]),
                "t": np.ascontiguousarray(tt[sl]),
                "table": tab,
            }
        )
    return in_maps


def _get_runner():
    """Build (once) a jitted shard_map callable over the 8 cores.

    Mirrors concourse.bass2jax.run_bass_via_pjrt, but caches the compiled
    executable so repeated kernel() calls skip retracing/recompiling, and
    allocates the donated output-zero buffers on device (no 205MB host
    transfer per call).
    """
    if "runner" in _cache:
        return _cache["runner"]

    import jax
    import jax.numpy as jnp
    from jax.sharding import Mesh, PartitionSpec
    from jax.experimental.shard_map import shard_map
    from concourse import mybir
    from concourse.bass2jax import (
        _bass_exec_p,
        install_neuronx_cc_hook,
        partition_id_tensor,
    )

    nc = _cache.get("nc")
    if nc is None:
        nc = _cache["nc"] = build_program()

    install_neuronx_cc_hook()

    partition_name = nc.partition_id_tensor.name if nc.partition_id_tensor else None
    in_names, out_names, out_avals = [], [], []
    for alloc in nc.m.functions[0].allocations:
        if not isinstance(alloc, mybir.MemoryLocationSet):
            continue
        name = alloc.memorylocations[0].name
        if alloc.kind == "ExternalInput":
            if name != partition_name:
                in_names.append(name)
        elif alloc.kind == "ExternalOutput":
            out_names.append(name)
            out_avals.append(
                jax.core.ShapedArray(tuple(alloc.tensor_shape), mybir.dt.np(alloc.dtype))
            )
    n_params = len(in_names)
    all_names = list(in_names) + out_names
    if partition_name is not None:
        all_names.append(partition_name)

    def _body(*args):
        # args = params + output placeholder buffers (the hook's parameter-
        # order check requires every bass_exec operand to be a jit parameter).
        operands = list(args)
        if partition_name is not None:
            operands.append(partition_id_tensor())
        outs = _bass_exec_p.bind(
            *operands,
            out_avals=tuple(out_avals),
            in_names=tuple(all_names),
            out_names=tuple(out_names),
            lowering_input_output_aliases=(),
            sim_require_finite=True,
            sim_require_nnan=True,
            nc=nc,
        )
        return tuple(outs)

    devices = jax.devices()[:N_CORES]
    assert len(devices) == N_CORES
    mesh = Mesh(np.asarray(devices), ("core",))
    n_outs = len(out_names)
    sharded = jax.jit(
        shard_map(
            _body,
            mesh=mesh,
            in_specs=(PartitionSpec("core"),) * (n_params + n_outs),
            out_specs=(PartitionSpec("core"),) * n_outs,
            check_rep=False,
        ),
        keep_unused=True,
    )
    # Output placeholder buffers: uploaded to device once, NOT donated, so
    # they stay valid and cost nothing on subsequent calls.
    from jax.sharding import NamedSharding

    zeros_dev = [
        jax.device_put(
            np.zeros((N_CORES * a.shape[0], *a.shape[1:]), a.dtype),
            NamedSharding(mesh, PartitionSpec("core")),
        )
        for a in out_avals
    ]
    _cache["runner"] = (sharded, in_names, out_names, zeros_dev)
    return _cache["runner"]


def kernel(images, e, t):
    images = np.asarray(images)
    orig_shape = images.shape

    x = np.ascontiguousarray(images.astype(np.float32, copy=False).reshape(B, PIX))
    yy = np.ascontiguousarray(
        np.asarray(e, dtype=np.float32).reshape(B, PIX)
    )
    tt = np.ascontiguousarray(np.asarray(t, dtype=np.int32).reshape(B))
    tab_global = np.tile(alpha_tables(), (N_CORES, 1))  # [8*T, 64]

    try:
        sharded, in_names, out_names, zeros_dev = _get_runner()
        global_in = {"x": x, "y": yy, "t": tt, "table": tab_global}
        out_arrs = sharded(*[global_in[n] for n in in_names], *zeros_dev)
        full = np.asarray(out_arrs[out_names.index("out")])
    except Exception:
        # Fallback: the stock (slower, but battle-tested) execution path.
        from concourse import bass_utils

        if "nc" not in _cache:
            _cache["nc"] = build_program()
        res = bass_utils.run_bass_kernel_spmd(
            _cache["nc"], make_in_maps(images, e, t), core_ids=list(range(N_CORES))
        )
        full = np.concatenate([res.results[c]["out"] for c in range(N_CORES)], axis=0)

    return full.reshape(orig_shape).astype(np.float32)
